# revision 13
# baseline (speedup 1.0000x reference)
"""Trainium2 Bass kernel for GQA causal self-attention.

Reference shapes: x [4, 2048, 2048], W_attn [2048, 3072] (q 16 heads | k 4 | v 4,
head_size 128), W_proj [2048, 2048]; causal softmax attention with GQA
(kv heads repeated 4x).

Sharding (8 cores): tensor-parallel over the 4 kv-head groups (each with its
4 query heads) x data-parallel over 2 batch groups (2 batches each). Each core:
  qkvT = W_loc.T @ x.T  (feature-major: features on partitions, tokens free)
  S^T[kt, qt] = kT.T @ qT per (batch, head); causal via additive -1e30 mask on
  diagonal 128-blocks, block-skip above the diagonal; exp on ACT; denominator
  accumulated on DVE then reduced with a ones-column fp32 matmul; 1/d broadcast
  to 128 partitions via a DRAM bounce; y^T = V.T-token-major @ E accumulated in
  PSUM; proj out[t, c] = yT.T @ Wp_loc partial, summed across head-group cores
  on the host.

All matmul operands are float32r (TF32-like, ~2e-4; full PE rate at N>=256).
"""

import math
import os

import ml_dtypes
import numpy as np

import concourse.bass as bass
import concourse.mybir as mybir
import concourse.tile as tile
from concourse.bass_utils import run_bass_kernel_spmd
from concourse.tile import add_dep_helper
from concourse.vector_clock import ScopedClock, VectorClock

F32 = mybir.dt.float32
F32R = mybir.dt.float32r
BF16 = mybir.dt.bfloat16
AF = mybir.ActivationFunctionType
ALU = mybir.AluOpType

N_HEAD = 16
N_KV = 4
HS = 128
C = 2048
T = 2048
B = 4
NQH = N_HEAD // N_KV          # q heads per kv group / per core
BG = 2                        # batch groups
B_LOC = B // BG               # batches per core
TLOC = B_LOC * T              # tokens per core
QKV_F = NQH * HS + 2 * HS     # local qkv feature dim (768)
KB = C // 128                 # K chunks (16)
KVD = N_KV * HS               # 512
SCALE = 1.0 / math.sqrt(HS)
NEG = -1.0e30

LAST_EXEC_NS = None


class _SplitDrainTileContext(tile.TileContext):
    """This walrus build accepts at most one sync wait on a Drain/NoOp
    (CTRL struct). The stock TileContext exit puts the whole global clock's
    waits on one Drain; emit carrier nops (one wait each) instead."""

    def _drain_and_barrier(self, tick_clock, wait_clock):
        gc = tick_clock.global_clock
        for p in range(len(gc)):
            if gc[p] <= 0:
                continue
            part = VectorClock([0] * len(gc))
            part.require_at_least(p, gc[p])
            nop_inst = self.nc.sync.nop(nofuse=True, hint="split_drain_wait")
            wait_clock.add_sem_waits(nop_inst.ins, ScopedClock({None: part}))
        self.nc.sync.drain()
        self.nc.all_engine_barrier()
        assert self.sems is not None
        popped = self.nc._tile_sem_poison_stack.pop()
        assert popped is self._sem_poison
        self.nc.clear_and_free_semaphores(list(self.sems.allocated().values()))
        self.nc.all_engine_barrier()


def _split_excess_waits(nc, max_keep=1):
    """Hoist all but ``max_keep`` sync waits from each instruction onto
    same-engine NoOps inserted just before it (walrus rejects e.g. two
    DMAHW waits on one Matmult's LDWEIGHTS struct)."""
    for func in nc.m.functions:
        for bb in func.blocks:
            insts = bb.instructions
            i = 0
            while i < len(insts):
                inst = insts[i]
                si = inst.sync_info
                waits = list(si.on_wait) if si is not None else []
                if len(waits) > max_keep:
                    keep = waits[-max_keep:]
                    for j, w in enumerate(waits[:-max_keep]):
                        nop = mybir.InstNoOp(
                            name=f"{inst.name}-wsplit{j}", ins=[], outs=[]
                        )
                        nop.engine = inst.engine
                        nop.sync_info = mybir.SyncInfo(on_wait=[w], on_update=[])
                        nc.register_instruction(nop, overwrite=True)
                        insts.insert(i, nop)
                        i += 1
                    inst.sync_info = mybir.SyncInfo(
                        on_wait=keep, on_update=list(si.on_update)
                    )
                i += 1


def _build_program(with_attn_bias):
    nc = bass.Bass()
    xT = nc.dram_tensor("xT", [C, TLOC], BF16, kind="ExternalInput")
    W = nc.dram_tensor("W", [C, QKV_F], BF16, kind="ExternalInput")
    Wp = nc.dram_tensor("Wp", [NQH * HS, C], BF16, kind="ExternalInput")
    tri = nc.dram_tensor("tri", [128, 128], F32, kind="ExternalInput")
    ident = nc.dram_tensor("ident", [128, 128], BF16, kind="ExternalInput")
    onescol = nc.dram_tensor("onescol", [128, 1], F32, kind="ExternalInput")
    if with_attn_bias:
        ba = nc.dram_tensor("ba", [1, QKV_F], BF16, kind="ExternalInput")
        onesrow = nc.dram_tensor("onesrow", [1, 512], BF16, kind="ExternalInput")
    out = nc.dram_tensor("out", [TLOC, C], F32, kind="ExternalOutput")
    # scratch rows for the 1/d partition-broadcast bounce, one per (b, qt, h)
    rscr = nc.dram_tensor("rscr", [B_LOC * (T // 512) * NQH, 512], F32)

    with _SplitDrainTileContext(nc) as tc:
        with (
            tc.tile_pool(name="consts", bufs=1) as consts,
            tc.tile_pool(name="wpool", bufs=1) as wpool,
            tc.tile_pool(name="xpool", bufs=2) as xpool,
            tc.tile_pool(name="qkv", bufs=2) as qkv,
            tc.tile_pool(name="epool", bufs=4) as epool,
            tc.tile_pool(name="dpool", bufs=2) as dpool,
            tc.tile_pool(name="ytpool", bufs=5) as ytpool,
            tc.tile_pool(name="rpool", bufs=2) as rpool,
            tc.tile_pool(name="opool", bufs=3) as opool,
            tc.tile_pool(name="psA", bufs=3, space="PSUM") as psA,
            tc.tile_pool(name="psS", bufs=1, space="PSUM") as psS,
            tc.tile_pool(name="psY", bufs=2, space="PSUM") as psY,
            tc.tile_pool(name="psD", bufs=2, space="PSUM") as psD,
        ):
            tri_t = consts.tile([128, 128], F32, tag="tri")
            ident_t = consts.tile([128, 128], BF16, tag="ident")
            ones_t = consts.tile([128, 1], F32, tag="onescol")
            nc.sync.dma_start(out=tri_t[:], in_=tri[:, :])
            nc.sync.dma_start(out=ident_t[:], in_=ident[:, :])
            nc.sync.dma_start(out=ones_t[:], in_=onescol[:, :])
            if with_attn_bias:
                ba_t = consts.tile([1, QKV_F], BF16, tag="ba")
                onesrow_t = consts.tile([1, 512], BF16, tag="onesrow")
                nc.sync.dma_start(out=ba_t[:], in_=ba[:, :])
                nc.sync.dma_start(out=onesrow_t[:], in_=onesrow[:, :])

            wk = []
            for k in range(KB):
                t = wpool.tile([128, QKV_F], BF16, tag=f"w{k}", name=f"w{k}")
                nc.sync.dma_start(out=t[:], in_=W[k * 128:(k + 1) * 128, :])
                wk.append(t)
            wpk = []
            NT = T // 512  # 512-token tiles per batch
            for b in range(B_LOC):
                qT = qkv.tile([128, NQH, T], BF16, tag="qT", name="qT")
                kT = qkv.tile([128, T], BF16, tag="kT", name="kT")
                vT = qkv.tile([128, T], BF16, tag="vT", name="vT")
                Vt = qkv.tile([128, T // 128, 128], BF16, tag="Vt", name="Vt")

                # ---- QKV projection (feature-major) ----
                for tt in range(NT):
                    t0 = b * T + tt * 512
                    xt = xpool.tile([128, KB, 512], BF16, tag="x", name="xt")
                    nc.sync.dma_start(
                        out=xt[:],
                        in_=xT[:, t0:t0 + 512].rearrange(
                            "(c p) t -> p c t", p=128
                        ),
                    )
                    for grp in range(3):
                        pfs = [
                            psA.tile([128, 512], F32, tag="a", name=f"qkvp{j}")
                            for j in range(2)
                        ]
                        for k in range(KB):
                            for j in range(2):
                                f = grp * 2 + j
                                nc.tensor.matmul(
                                    out=pfs[j][:],
                                    lhsT=wk[k][:, f * 128:(f + 1) * 128],
                                    rhs=xt[:, k, :],
                                    start=(k == 0),
                                    stop=(k == KB - 1 and not with_attn_bias),
                                )
                        if with_attn_bias:
                            for j in range(2):
                                f = grp * 2 + j
                                nc.tensor.matmul(
                                    out=pfs[j][:],
                                    lhsT=ba_t[0:1, f * 128:(f + 1) * 128],
                                    rhs=onesrow_t[0:1, :],
                                    start=False,
                                    stop=True,
                                )
                        for j in range(2):
                            f = grp * 2 + j
                            if f < NQH:
                                dest = qT[:, f, tt * 512:(tt + 1) * 512]
                            elif f == NQH:
                                dest = kT[:, tt * 512:(tt + 1) * 512]
                            else:
                                dest = vT[:, tt * 512:(tt + 1) * 512]
                            nc.scalar.copy(out=dest, in_=pfs[j][:])

                if not wpk:
                    for j in range(NQH):
                        t = wpool.tile(
                            [128, C], BF16, tag=f"wp{j}", name=f"wp{j}"
                        )
                        nc.sync.dma_start(
                            out=t[:], in_=Wp[j * 128:(j + 1) * 128, :]
                        )
                        wpk.append(t)

                # ---- V to token-major via PE transpose ----
                for s in range(T // 128):
                    pt = psA.tile([128, 128], BF16, tag="a", name="vtp")
                    nc.tensor.transpose(
                        out=pt[:], in_=vT[:, s * 128:(s + 1) * 128], identity=ident_t[:]
                    )
                    nc.vector.tensor_copy(out=Vt[:, s, :], in_=pt[:])

                # ---- attention + out-proj per 512-token query tile ----
                for qt in range(NT):
                    q0 = qt * 512
                    yts = []
                    for h in range(NQH):
                        y_ps = psY.tile([128, 512], F32, tag="y", name="y_ps")
                        d_acc = dpool.tile([128, 512], F32, tag="d", name="d_acc")
                        nkt = 4 * qt + 4
                        for kt in range(nkt):
                            dl = kt * 128 - q0
                            w0 = max(dl, 0)
                            s_ps = psS.tile([128, 512], F32, tag="s", name="s_ps")
                            nc.tensor.matmul(
                                out=s_ps[:, w0:512],
                                lhsT=kT[:, kt * 128:(kt + 1) * 128],
                                rhs=qT[:, h, q0 + w0:q0 + 512],
                                start=True,
                                stop=True,
                            )
                            if dl >= 0:
                                nc.vector.tensor_tensor(
                                    out=s_ps[:, dl:dl + 128],
                                    in0=s_ps[:, dl:dl + 128],
                                    in1=tri_t[:],
                                    op=ALU.add,
                                )
                            e = epool.tile([128, 512], BF16, tag="e", name="e")
                            nc.scalar.activation(
                                out=e[:, w0:512],
                                in_=s_ps[:, w0:512],
                                func=AF.Exp,
                                scale=SCALE,
                            )
                            if kt == 0:
                                nc.vector.tensor_copy(
                                    out=d_acc[:], in_=e[:]
                                )
                            else:
                                nc.vector.tensor_tensor(
                                    out=d_acc[:, w0:512],
                                    in0=d_acc[:, w0:512],
                                    in1=e[:, w0:512],
                                    op=ALU.add,
                                )
                            nc.tensor.matmul(
                                out=y_ps[:, w0:512],
                                lhsT=Vt[:, kt, :],
                                rhs=e[:, w0:512],
                                start=(kt == 0),
                                stop=(kt == nkt - 1),
                            )
                        # free the y PSUM bank quickly: copy unnormalized
                        yt_raw = ytpool.tile(
                            [128, 512], BF16, tag="ytr", name="yt_raw"
                        )
                        nc.vector.tensor_copy(out=yt_raw[:], in_=y_ps[:])
                        # transposed denominator: dT[qt_sub 128, 4] so the
                        # reciprocal runs 4 elem/lane instead of 512 on 1 lane
                        dT_ps = psD.tile([128, 4], F32, tag="dp", name="dT_ps")
                        d_acc_r = d_acc[:].rearrange("k (q s) -> k s q", s=4)
                        for s in range(4):
                            nc.tensor.matmul(
                                out=dT_ps[:, s:s + 1],
                                lhsT=d_acc_r[:, s, :],
                                rhs=ones_t[:, 0:1],
                                start=(s == 0),
                                stop=(s == 3),
                            )
                        rT_sb = rpool.tile([128, 4], F32, tag="r", name="rT_sb")
                        nc.vector.reciprocal(out=rT_sb[:], in_=dT_ps[:])
                        row = (b * NT + qt) * NQH + h
                        w_inst = nc.sync.dma_start(
                            out=bass.AP(
                                tensor=rscr[0].tensor,
                                offset=row * 512,
                                ap=[[4, 128], [1, 4]],
                            ),
                            in_=rT_sb[:],
                        )
                        rb = rpool.tile([128, 512], F32, tag="rb", name="rb")
                        r_bcast = bass.AP(
                            tensor=rscr[0].tensor,
                            offset=row * 512,
                            ap=[[0, 128], [1, 512]],
                        )
                        rd_inst = nc.sync.dma_start(out=rb[:], in_=r_bcast)
                        add_dep_helper(
                            rd_inst.ins, w_inst.ins, sync=True,
                            reason="rscr bounce RAW",
                        )
                        yt = ytpool.tile([128, 512], BF16, tag="yt", name="yt")
                        nc.vector.tensor_tensor(
                            out=yt[:], in0=yt_raw[:], in1=rb[:], op=ALU.mult
                        )
                        yts.append(yt)
                    # out projection for this query tile (sums local heads)
                    for ts in range(4):
                        o_sb = opool.tile([128, C], F32, tag="o", name="o_sb")
                        for ct in range(C // 512):
                            o_ps = psA.tile([128, 512], F32, tag="a", name="o_ps")
                            for h in range(NQH):
                                nc.tensor.matmul(
                                    out=o_ps[:],
                                    lhsT=yts[h][:, ts * 128:(ts + 1) * 128],
                                    rhs=wpk[h][:, ct * 512:(ct + 1) * 512],
                                    start=(h == 0),
                                    stop=(h == NQH - 1),
                                )
                            if ct % 2 == 0:
                                nc.scalar.copy(
                                    out=o_sb[:, ct * 512:(ct + 1) * 512],
                                    in_=o_ps[:],
                                )
                            else:
                                nc.vector.tensor_copy(
                                    out=o_sb[:, ct * 512:(ct + 1) * 512],
                                    in_=o_ps[:],
                                )
                        nc.sync.dma_start(
                            out=out[
                                b * T + q0 + ts * 128:b * T + q0 + ts * 128 + 128, :
                            ],
                            in_=o_sb[:],
                        )

    _split_excess_waits(nc)
    return nc


_prog_cache = {}


def _get_program(with_attn_bias):
    key = bool(with_attn_bias)
    if key not in _prog_cache:
        _prog_cache[key] = _build_program(key)
    return _prog_cache[key]


def kernel(x, W_attn, b_attn, W_proj, b_proj):
    global LAST_EXEC_NS
    x = np.ascontiguousarray(np.asarray(x, dtype=np.float32))
    W_attn = np.asarray(W_attn, dtype=np.float32)
    b_attn = np.asarray(b_attn, dtype=np.float32)
    W_proj = np.asarray(W_proj, dtype=np.float32)
    b_proj = np.asarray(b_proj, dtype=np.float32)

    with_attn_bias = bool(np.any(b_attn))
    nc = _get_program(with_attn_bias)

    tri = np.where(
        np.arange(128)[:, None] <= np.arange(128)[None, :], 0.0, NEG
    ).astype(np.float32)
    ident = np.eye(128, dtype=np.float32)
    onescol = np.ones((128, 1), np.float32)

    in_maps = [None] * (BG * N_KV)
    for bg in range(BG):
        xT_bg = np.ascontiguousarray(
            x[B_LOC * bg:B_LOC * (bg + 1)].reshape(TLOC, C).T
        ).astype(ml_dtypes.bfloat16)
        for g in range(N_KV):
            W_loc = np.ascontiguousarray(
                np.concatenate(
                    [
                        W_attn[:, g * NQH * HS:(g + 1) * NQH * HS],
                        W_attn[:, C + g * HS:C + (g + 1) * HS],
                        W_attn[:, C + KVD + g * HS:C + KVD + (g + 1) * HS],
                    ],
                    axis=1,
                )
            )
            Wp_loc = np.ascontiguousarray(
                W_proj[g * NQH * HS:(g + 1) * NQH * HS, :]
            )
            m = dict(
                xT=xT_bg, W=W_loc.astype(ml_dtypes.bfloat16),
                Wp=Wp_loc.astype(ml_dtypes.bfloat16), tri=tri,
                ident=ident.astype(ml_dtypes.bfloat16), onescol=onescol,
            )
            if with_attn_bias:
                ba_loc = np.concatenate(
                    [
                        b_attn[g * NQH * HS:(g + 1) * NQH * HS],
                        b_attn[C + g * HS:C + (g + 1) * HS],
                        b_attn[C + KVD + g * HS:C + KVD + (g + 1) * HS],
                    ]
                )
                m["ba"] = np.ascontiguousarray(ba_loc[None, :]).astype(ml_dtypes.bfloat16)
                m["onesrow"] = np.ones((1, 512), ml_dtypes.bfloat16)
            in_maps[bg * N_KV + g] = m

    trace = os.environ.get("KERNEL_TRACE") == "1"
    kwargs = {}
    if trace:
        kwargs = dict(trace=True, trace_cores=[0])
    res = run_bass_kernel_spmd(
        nc, in_maps, core_ids=list(range(BG * N_KV)), **kwargs
    )
    LAST_EXEC_NS = res.exec_time_ns

    out = np.empty((B, T, C), np.float32)
    for bg in range(BG):
        acc = res.results[bg * N_KV + 0]["out"].copy()
        for g in range(1, N_KV):
            acc += res.results[bg * N_KV + g]["out"]
        if np.any(b_proj):
            acc += b_proj[None, :]
        out[B_LOC * bg:B_LOC * (bg + 1)] = acc.reshape(B_LOC, T, C)
    return out


# revision 14
# speedup vs baseline: 1.3996x; 1.3996x over previous
"""Trainium2 Bass kernel for GQA causal self-attention.

Reference shapes: x [4, 2048, 2048], W_attn [2048, 3072] (q 16 heads | k 4 | v 4,
head_size 128), W_proj [2048, 2048]; causal softmax attention with GQA
(kv heads repeated 4x).

Sharding (8 cores): tensor-parallel over the 4 kv-head groups (each with its
4 query heads) x data-parallel over 2 batch groups (2 batches each). Each core:
  qkvT = W_loc.T @ x.T  (feature-major: features on partitions, tokens free)
  S^T[kt, qt] = kT.T @ qT per (batch, head); causal via additive -1e30 mask on
  diagonal 128-blocks, block-skip above the diagonal; exp on ACT; denominator
  accumulated on DVE then reduced with a ones-column fp32 matmul; 1/d broadcast
  to 128 partitions via a DRAM bounce; y^T = V.T-token-major @ E accumulated in
  PSUM; proj out[t, c] = yT.T @ Wp_loc partial, summed across head-group cores
  on the host.

All matmul operands are float32r (TF32-like, ~2e-4; full PE rate at N>=256).
"""

import math
import os

import ml_dtypes
import numpy as np

import concourse.bass as bass
import concourse.mybir as mybir
import concourse.tile as tile
from concourse.bass_utils import run_bass_kernel_spmd
from concourse.tile import add_dep_helper
from concourse.vector_clock import ScopedClock, VectorClock

F32 = mybir.dt.float32
F32R = mybir.dt.float32r
BF16 = mybir.dt.bfloat16
AF = mybir.ActivationFunctionType
ALU = mybir.AluOpType

N_HEAD = 16
N_KV = 4
HS = 128
C = 2048
T = 2048
B = 4
NQH = N_HEAD // N_KV          # q heads per kv group / per core
BG = 2                        # batch groups
B_LOC = B // BG               # batches per core
TLOC = B_LOC * T              # tokens per core
QKV_F = NQH * HS + 2 * HS     # local qkv feature dim (768)
KB = C // 128                 # K chunks (16)
KVD = N_KV * HS               # 512
SCALE = 1.0 / math.sqrt(HS)
NEG = -1.0e30

LAST_EXEC_NS = None


class _SplitDrainTileContext(tile.TileContext):
    """This walrus build accepts at most one sync wait on a Drain/NoOp
    (CTRL struct). The stock TileContext exit puts the whole global clock's
    waits on one Drain; emit carrier nops (one wait each) instead."""

    def _drain_and_barrier(self, tick_clock, wait_clock):
        gc = tick_clock.global_clock
        for p in range(len(gc)):
            if gc[p] <= 0:
                continue
            part = VectorClock([0] * len(gc))
            part.require_at_least(p, gc[p])
            nop_inst = self.nc.sync.nop(nofuse=True, hint="split_drain_wait")
            wait_clock.add_sem_waits(nop_inst.ins, ScopedClock({None: part}))
        self.nc.sync.drain()
        self.nc.all_engine_barrier()
        assert self.sems is not None
        popped = self.nc._tile_sem_poison_stack.pop()
        assert popped is self._sem_poison
        self.nc.clear_and_free_semaphores(list(self.sems.allocated().values()))
        self.nc.all_engine_barrier()


def _split_excess_waits(nc, max_keep=1):
    """Hoist all but ``max_keep`` sync waits from each instruction onto
    same-engine NoOps inserted just before it (walrus rejects e.g. two
    DMAHW waits on one Matmult's LDWEIGHTS struct)."""
    for func in nc.m.functions:
        for bb in func.blocks:
            insts = bb.instructions
            i = 0
            while i < len(insts):
                inst = insts[i]
                si = inst.sync_info
                waits = list(si.on_wait) if si is not None else []
                if len(waits) > max_keep:
                    keep = waits[-max_keep:]
                    for j, w in enumerate(waits[:-max_keep]):
                        nop = mybir.InstNoOp(
                            name=f"{inst.name}-wsplit{j}", ins=[], outs=[]
                        )
                        nop.engine = inst.engine
                        nop.sync_info = mybir.SyncInfo(on_wait=[w], on_update=[])
                        nc.register_instruction(nop, overwrite=True)
                        insts.insert(i, nop)
                        i += 1
                    inst.sync_info = mybir.SyncInfo(
                        on_wait=keep, on_update=list(si.on_update)
                    )
                i += 1


def _build_program(with_attn_bias):
    nc = bass.Bass()
    xT = nc.dram_tensor("xT", [C, TLOC], BF16, kind="ExternalInput")
    W = nc.dram_tensor("W", [C, QKV_F], BF16, kind="ExternalInput")
    Wp = nc.dram_tensor("Wp", [NQH * HS, C], BF16, kind="ExternalInput")
    tri = nc.dram_tensor("tri", [128, 128], F32, kind="ExternalInput")
    ident = nc.dram_tensor("ident", [128, 128], BF16, kind="ExternalInput")
    onescol = nc.dram_tensor("onescol", [128, 1], F32, kind="ExternalInput")
    if with_attn_bias:
        ba = nc.dram_tensor("ba", [1, QKV_F], BF16, kind="ExternalInput")
        onesrow = nc.dram_tensor("onesrow", [1, 512], BF16, kind="ExternalInput")
    out = nc.dram_tensor("out", [TLOC, C], F32, kind="ExternalOutput")
    # scratch rows for the 1/d partition-broadcast bounce, one per (b, qt, h)
    rscr = nc.dram_tensor("rscr", [B_LOC * (T // 512) * NQH, 512], F32)

    with _SplitDrainTileContext(nc) as tc:
        with (
            tc.tile_pool(name="consts", bufs=1) as consts,
            tc.tile_pool(name="wpool", bufs=1) as wpool,
            tc.tile_pool(name="xpool", bufs=2) as xpool,
            tc.tile_pool(name="qkv", bufs=2) as qkv,
            tc.tile_pool(name="epool", bufs=4) as epool,
            tc.tile_pool(name="dpool", bufs=2) as dpool,
            tc.tile_pool(name="ytpool", bufs=5) as ytpool,
            tc.tile_pool(name="rpool", bufs=2) as rpool,
            tc.tile_pool(name="opool", bufs=3) as opool,
            tc.tile_pool(name="psA", bufs=3, space="PSUM") as psA,
            tc.tile_pool(name="psS", bufs=2, space="PSUM") as psS,
            tc.tile_pool(name="psY", bufs=2, space="PSUM") as psY,
            tc.tile_pool(name="psD", bufs=1, space="PSUM") as psD,
        ):
            tri_t = consts.tile([128, 128], F32, tag="tri")
            ident_t = consts.tile([128, 128], BF16, tag="ident")
            ones_t = consts.tile([128, 1], F32, tag="onescol")
            nc.sync.dma_start(out=tri_t[:], in_=tri[:, :])
            nc.sync.dma_start(out=ident_t[:], in_=ident[:, :])
            nc.sync.dma_start(out=ones_t[:], in_=onescol[:, :])
            if with_attn_bias:
                ba_t = consts.tile([1, QKV_F], BF16, tag="ba")
                onesrow_t = consts.tile([1, 512], BF16, tag="onesrow")
                nc.sync.dma_start(out=ba_t[:], in_=ba[:, :])
                nc.sync.dma_start(out=onesrow_t[:], in_=onesrow[:, :])

            wk = []
            for k in range(KB):
                t = wpool.tile([128, QKV_F], BF16, tag=f"w{k}", name=f"w{k}")
                nc.sync.dma_start(out=t[:], in_=W[k * 128:(k + 1) * 128, :])
                wk.append(t)
            wpk = []
            NT = T // 512  # 512-token tiles per batch
            for b in range(B_LOC):
                qT = qkv.tile([128, NQH, T], BF16, tag="qT", name="qT")
                kT = qkv.tile([128, T], BF16, tag="kT", name="kT")
                vT = qkv.tile([128, T], BF16, tag="vT", name="vT")
                Vt = qkv.tile([128, T // 128, 128], BF16, tag="Vt", name="Vt")

                # ---- QKV projection (feature-major) ----
                for tt in range(NT):
                    t0 = b * T + tt * 512
                    xt = xpool.tile([128, KB, 512], BF16, tag="x", name="xt")
                    nc.sync.dma_start(
                        out=xt[:],
                        in_=xT[:, t0:t0 + 512].rearrange(
                            "(c p) t -> p c t", p=128
                        ),
                    )
                    for grp in range(3):
                        pfs = [
                            psA.tile([128, 512], F32, tag="a", name=f"qkvp{j}")
                            for j in range(2)
                        ]
                        for k in range(KB):
                            for j in range(2):
                                f = grp * 2 + j
                                nc.tensor.matmul(
                                    out=pfs[j][:],
                                    lhsT=wk[k][:, f * 128:(f + 1) * 128],
                                    rhs=xt[:, k, :],
                                    start=(k == 0),
                                    stop=(k == KB - 1 and not with_attn_bias),
                                )
                        if with_attn_bias:
                            for j in range(2):
                                f = grp * 2 + j
                                nc.tensor.matmul(
                                    out=pfs[j][:],
                                    lhsT=ba_t[0:1, f * 128:(f + 1) * 128],
                                    rhs=onesrow_t[0:1, :],
                                    start=False,
                                    stop=True,
                                )
                        for j in range(2):
                            f = grp * 2 + j
                            if f < NQH:
                                dest = qT[:, f, tt * 512:(tt + 1) * 512]
                            elif f == NQH:
                                dest = kT[:, tt * 512:(tt + 1) * 512]
                            else:
                                dest = vT[:, tt * 512:(tt + 1) * 512]
                            nc.scalar.copy(out=dest, in_=pfs[j][:])

                if not wpk:
                    for j in range(NQH):
                        t = wpool.tile(
                            [128, C], BF16, tag=f"wp{j}", name=f"wp{j}"
                        )
                        nc.sync.dma_start(
                            out=t[:], in_=Wp[j * 128:(j + 1) * 128, :]
                        )
                        wpk.append(t)

                # ---- V to token-major via PE transpose ----
                for s in range(T // 128):
                    pt = psA.tile([128, 128], BF16, tag="a", name="vtp")
                    nc.tensor.transpose(
                        out=pt[:], in_=vT[:, s * 128:(s + 1) * 128], identity=ident_t[:]
                    )
                    nc.vector.tensor_copy(out=Vt[:, s, :], in_=pt[:])

                # ---- attention + out-proj per 512-token query tile ----
                for qt in range(NT):
                    q0 = qt * 512
                    yts = []
                    for h in range(NQH):
                        y_ps = psY.tile([128, 512], F32, tag="y", name="y_ps")
                        d_acc = dpool.tile([128, 512], F32, tag="d", name="d_acc")
                        nkt = 4 * qt + 4
                        for kt in range(nkt):
                            dl = kt * 128 - q0
                            w0 = max(dl, 0)
                            s_ps = psS.tile([128, 512], F32, tag="s", name="s_ps")
                            nc.tensor.matmul(
                                out=s_ps[:, w0:512],
                                lhsT=kT[:, kt * 128:(kt + 1) * 128],
                                rhs=qT[:, h, q0 + w0:q0 + 512],
                                start=True,
                                stop=True,
                            )
                            if dl >= 0:
                                nc.vector.tensor_tensor(
                                    out=s_ps[:, dl:dl + 128],
                                    in0=s_ps[:, dl:dl + 128],
                                    in1=tri_t[:],
                                    op=ALU.add,
                                )
                            e = epool.tile([128, 512], BF16, tag="e", name="e")
                            nc.scalar.activation(
                                out=e[:, w0:512],
                                in_=s_ps[:, w0:512],
                                func=AF.Exp,
                                scale=SCALE,
                            )
                            if kt == 0:
                                nc.vector.tensor_copy(
                                    out=d_acc[:], in_=e[:]
                                )
                            else:
                                nc.vector.tensor_tensor(
                                    out=d_acc[:, w0:512],
                                    in0=d_acc[:, w0:512],
                                    in1=e[:, w0:512],
                                    op=ALU.add,
                                )
                            nc.tensor.matmul(
                                out=y_ps[:, w0:512],
                                lhsT=Vt[:, kt, :],
                                rhs=e[:, w0:512],
                                start=(kt == 0),
                                stop=(kt == nkt - 1),
                            )
                        # free the y PSUM bank quickly: copy unnormalized
                        yt_raw = ytpool.tile(
                            [128, 512], BF16, tag="ytr", name="yt_raw"
                        )
                        nc.vector.tensor_copy(out=yt_raw[:], in_=y_ps[:])
                        # transposed denominator: dT[qt_sub 128, 4] so the
                        # reciprocal runs 4 elem/lane instead of 512 on 1 lane
                        dT_ps = psD.tile([128, 4], F32, tag="dp", name="dT_ps")
                        d_acc_r = d_acc[:].rearrange("k (q s) -> k s q", s=4)
                        for s in range(4):
                            nc.tensor.matmul(
                                out=dT_ps[:, s:s + 1],
                                lhsT=d_acc_r[:, s, :],
                                rhs=ones_t[:, 0:1],
                                start=(s == 0),
                                stop=(s == 3),
                            )
                        rT_sb = rpool.tile([128, 4], F32, tag="r", name="rT_sb")
                        nc.vector.reciprocal(out=rT_sb[:], in_=dT_ps[:])
                        row = (b * NT + qt) * NQH + h
                        w_inst = nc.sync.dma_start(
                            out=bass.AP(
                                tensor=rscr[0].tensor,
                                offset=row * 512,
                                ap=[[4, 128], [1, 4]],
                            ),
                            in_=rT_sb[:],
                        )
                        rb = rpool.tile([128, 512], F32, tag="rb", name="rb")
                        r_bcast = bass.AP(
                            tensor=rscr[0].tensor,
                            offset=row * 512,
                            ap=[[0, 128], [1, 512]],
                        )
                        rd_inst = nc.sync.dma_start(out=rb[:], in_=r_bcast)
                        add_dep_helper(
                            rd_inst.ins, w_inst.ins, sync=True,
                            reason="rscr bounce RAW",
                        )
                        yt = ytpool.tile([128, 512], BF16, tag="yt", name="yt")
                        nc.vector.tensor_tensor(
                            out=yt[:], in0=yt_raw[:], in1=rb[:], op=ALU.mult
                        )
                        yts.append(yt)
                    # out projection for this query tile (sums local heads)
                    for ts in range(4):
                        o_sb = opool.tile([128, C], F32, tag="o", name="o_sb")
                        for ct in range(C // 512):
                            o_ps = psA.tile([128, 512], F32, tag="a", name="o_ps")
                            for h in range(NQH):
                                nc.tensor.matmul(
                                    out=o_ps[:],
                                    lhsT=yts[h][:, ts * 128:(ts + 1) * 128],
                                    rhs=wpk[h][:, ct * 512:(ct + 1) * 512],
                                    start=(h == 0),
                                    stop=(h == NQH - 1),
                                )
                            if ct % 2 == 0:
                                nc.scalar.copy(
                                    out=o_sb[:, ct * 512:(ct + 1) * 512],
                                    in_=o_ps[:],
                                )
                            else:
                                nc.vector.tensor_copy(
                                    out=o_sb[:, ct * 512:(ct + 1) * 512],
                                    in_=o_ps[:],
                                )
                        nc.sync.dma_start(
                            out=out[
                                b * T + q0 + ts * 128:b * T + q0 + ts * 128 + 128, :
                            ],
                            in_=o_sb[:],
                        )

    _split_excess_waits(nc)
    return nc


_prog_cache = {}


def _get_program(with_attn_bias):
    key = bool(with_attn_bias)
    if key not in _prog_cache:
        _prog_cache[key] = _build_program(key)
    return _prog_cache[key]


def kernel(x, W_attn, b_attn, W_proj, b_proj):
    global LAST_EXEC_NS
    x = np.ascontiguousarray(np.asarray(x, dtype=np.float32))
    W_attn = np.asarray(W_attn, dtype=np.float32)
    b_attn = np.asarray(b_attn, dtype=np.float32)
    W_proj = np.asarray(W_proj, dtype=np.float32)
    b_proj = np.asarray(b_proj, dtype=np.float32)

    with_attn_bias = bool(np.any(b_attn))
    nc = _get_program(with_attn_bias)

    tri = np.where(
        np.arange(128)[:, None] <= np.arange(128)[None, :], 0.0, NEG
    ).astype(np.float32)
    ident = np.eye(128, dtype=np.float32)
    onescol = np.ones((128, 1), np.float32)

    in_maps = [None] * (BG * N_KV)
    for bg in range(BG):
        xT_bg = np.ascontiguousarray(
            x[B_LOC * bg:B_LOC * (bg + 1)].reshape(TLOC, C).T
        ).astype(ml_dtypes.bfloat16)
        for g in range(N_KV):
            W_loc = np.ascontiguousarray(
                np.concatenate(
                    [
                        W_attn[:, g * NQH * HS:(g + 1) * NQH * HS],
                        W_attn[:, C + g * HS:C + (g + 1) * HS],
                        W_attn[:, C + KVD + g * HS:C + KVD + (g + 1) * HS],
                    ],
                    axis=1,
                )
            )
            Wp_loc = np.ascontiguousarray(
                W_proj[g * NQH * HS:(g + 1) * NQH * HS, :]
            )
            m = dict(
                xT=xT_bg, W=W_loc.astype(ml_dtypes.bfloat16),
                Wp=Wp_loc.astype(ml_dtypes.bfloat16), tri=tri,
                ident=ident.astype(ml_dtypes.bfloat16), onescol=onescol,
            )
            if with_attn_bias:
                ba_loc = np.concatenate(
                    [
                        b_attn[g * NQH * HS:(g + 1) * NQH * HS],
                        b_attn[C + g * HS:C + (g + 1) * HS],
                        b_attn[C + KVD + g * HS:C + KVD + (g + 1) * HS],
                    ]
                )
                m["ba"] = np.ascontiguousarray(ba_loc[None, :]).astype(ml_dtypes.bfloat16)
                m["onesrow"] = np.ones((1, 512), ml_dtypes.bfloat16)
            in_maps[bg * N_KV + g] = m

    trace = os.environ.get("KERNEL_TRACE") == "1"
    kwargs = {}
    if trace:
        kwargs = dict(trace=True, trace_cores=[0])
    res = run_bass_kernel_spmd(
        nc, in_maps, core_ids=list(range(BG * N_KV)), **kwargs
    )
    LAST_EXEC_NS = res.exec_time_ns

    out = np.empty((B, T, C), np.float32)
    for bg in range(BG):
        acc = res.results[bg * N_KV + 0]["out"].copy()
        for g in range(1, N_KV):
            acc += res.results[bg * N_KV + g]["out"]
        if np.any(b_proj):
            acc += b_proj[None, :]
        out[B_LOC * bg:B_LOC * (bg + 1)] = acc.reshape(B_LOC, T, C)
    return out
